# revision 20
# baseline (speedup 1.0000x reference)
"""Trainium2 Bass kernel for EquivariantAttention (sparse_attention).

Full (unsharded) inputs in, full output out. Internally shards over the 8
NeuronCores as (batch, T-half): core c handles batch b = c // 2, query rows
t0 = (c % 2) * 256 .. t0+256.  Every core runs the identical SPMD program on
its own input slices; there is no cross-core communication (LN and out_proj
are row-local in (b, t)).

Device-side per core:
  scores_T[s,(i,t)] = bias_T + (k_T.T @ q_T)      (bias preloaded into PSUM
                                                   via identity matmul, QK
                                                   accumulates on top)
  m = (scores_T + 20) * law_T                     (one DVE pass, fp32)
  e = exp(m - 20)  -> bf16                        (ACT, free affine bias)
  g = e * law_T                                   (DVE/gpsimd, bf16)
  den[i,t] = sum_s e                              (4 col-tiled ones-matmuls,
                                                   rows 32i of one PSUM bank)
  attn_ps[(i,dd),(p,t)] += v[s,(h,p,dd)].T @ g    (col-tiled matmuls write
                                                   channel-major attn directly
                                                   -> no head remap phase)
  attn_ct = attn_ps * recip(den)                  (divide fused into the PSUM
                                                   eviction, bcast via SBUF DMA)
  ssq accumulated via ones-matmuls on attn_ct^2   (ACT square)
  out = (attn_ct @ (out_proj_w * ln_w).T) * inorm (inorm cols via K=1 PE
                                                   broadcast, applied on ACT
                                                   PSUM->SBUF copy-out)
"""

import numpy as np
import ml_dtypes

import concourse.bass as bass
import concourse.bacc as bacc
import concourse.tile as tile
from concourse import mybir
from concourse.bass_utils import run_bass_kernel_spmd

# Problem constants (hardcoded per contract)
B, T, P, HID = 4, 512, 3, 512
H, D = 16, 32
EXP = 256
S = T + EXP            # 768
SCALING = (D / 3.0) ** 0.5 / D
SMOOTH = 20.0
EPS = 1e-3

NCORES = 8
TQ = T // 2            # 256 query rows per core
DH = P * D             # 96 head dim
NST = S // 128         # 6 s-tiles of 128
HG = 4                 # head groups of 4 heads

F32 = mybir.dt.float32
F16 = mybir.dt.float16
F32R = mybir.dt.float32r
BF16 = mybir.dt.bfloat16
I32 = mybir.dt.int32
AF = mybir.ActivationFunctionType
ALU = mybir.AluOpType

_CACHED_NC = None


def build_nc():
    nc = bacc.Bacc("TRN2", target_bir_lowering=False, debug=False)

    # ---- DRAM I/O (per-core shapes) ----
    d_bias = nc.dram_tensor("biasT", [S, H, TQ], F16, kind="ExternalInput").ap()
    d_law = nc.dram_tensor("lawT", [128, NST * TQ], F32, kind="ExternalInput").ap()
    d_lawb = nc.dram_tensor("lawTb", [128, NST * TQ], BF16, kind="ExternalInput").ap()
    d_qT = nc.dram_tensor("qT", [DH, H * TQ], BF16, kind="ExternalInput").ap()
    # K columns reordered (st, h, sd) so each (hg, st) chunk is contiguous
    d_kst = nc.dram_tensor("kst", [DH, NST * H * 128], BF16, kind="ExternalInput").ap()
    d_vb = nc.dram_tensor("vb", [T, P * HID], BF16, kind="ExternalInput").ap()
    d_vidx = nc.dram_tensor("vidx", [2, 128, 1], I32, kind="ExternalInput").ap()
    d_wT = nc.dram_tensor("wT", [HID, HID], F16, kind="ExternalInput").ap()
    d_id = nc.dram_tensor("ident", [128, 128], F16, kind="ExternalInput").ap()
    d_out = nc.dram_tensor("out", [TQ, P, HID], F32, kind="ExternalOutput").ap()
    d_rec = nc.dram_tensor("rec_scratch", [HG, 4, TQ], F32).ap()

    with tile.TileContext(nc) as tc:
        build_kernel(tc, d_bias, d_law, d_lawb, d_qT, d_kst, d_vb, d_vidx,
                     d_wT, d_id, d_out, d_rec)
    nc.compile()
    return nc


def build_kernel(tc, d_bias, d_law, d_lawb, d_qT, d_kst, d_vb, d_vidx,
                 d_wT, d_id, d_out, d_rec):
    nc = tc.nc
    from contextlib import ExitStack, nullcontext
    ctx = ExitStack()
    with ctx:
        const = ctx.enter_context(tc.tile_pool(name="const", bufs=1))
        big = ctx.enter_context(tc.tile_pool(name="big", bufs=1))
        biasp = ctx.enter_context(tc.tile_pool(name="biasp", bufs=8))
        work = ctx.enter_context(tc.tile_pool(name="work", bufs=3))
        attnp = ctx.enter_context(tc.tile_pool(name="attnp", bufs=1))
        psum = ctx.enter_context(tc.tile_pool(name="psum", bufs=2, space="PSUM"))
        psum1 = ctx.enter_context(tc.tile_pool(name="psum1", bufs=1, space="PSUM"))

        # ---- constants ----
        ident = const.tile([128, 128], F16, tag="ident")
        ones_b = const.tile([128, 1], BF16, tag="ones_b")
        ones_f = const.tile([128, 1], F32, tag="ones_f")
        neg20 = const.tile([128, 1], F32, tag="neg20")
        zeros = const.tile([128, 512], F16, tag="zeros")
        nc.vector.memset(ones_b[:], 1.0)
        nc.vector.memset(ones_f[:], 1.0)
        nc.vector.memset(neg20[:], -SMOOTH)
        nc.vector.memset(zeros[:], 0.0)

        # ---- resident tiles ----
        law = const.tile([128, NST * TQ], F32, tag="law")      # (s%128,(st,t))
        lawb = const.tile([128, NST * TQ], BF16, tag="lawb")
        qT = const.tile([DH, H * TQ], BF16, tag="qT")
        kst = big.tile([DH, NST * H * 128], BF16, tag="kst")   # (d,(st,h,sd))
        wT = const.tile([128, 4 * HID], F16, tag="wT")         # (c%128,(ci,o))

        def kchunk(hg, st):
            lo = st * (H * 128) + hg * 512
            return (kst[:, lo:lo + 512], d_kst[:, lo:lo + 512])

        # need-ordered startup: first-tile inputs at high priority
        with tc.high_priority():
            nc.sync.dma_start(out=ident[:], in_=d_id)
            nc.sync.dma_start(out=law[:, :TQ], in_=d_law[:, :TQ])
            s_, d_ = kchunk(0, 0)
            nc.sync.dma_start(out=s_, in_=d_)
            nc.sync.dma_start(out=qT[:, :4 * TQ], in_=d_qT[:, :4 * TQ])
        # remaining law / lawb chunks
        for st in range(1, NST):
            nc.sync.dma_start(out=law[:, st * TQ:(st + 1) * TQ],
                              in_=d_law[:, st * TQ:(st + 1) * TQ])
        for st in range(NST):
            nc.sync.dma_start(out=lawb[:, st * TQ:(st + 1) * TQ],
                              in_=d_lawb[:, st * TQ:(st + 1) * TQ])
        # remaining K chunks: hg-major so hg0's s-tiles all arrive first
        for hg in range(HG):
            for st in range(NST):
                if hg == 0 and st == 0:
                    continue
                s_, d_ = kchunk(hg, st)
                nc.sync.dma_start(out=s_, in_=d_)
            if hg < HG - 1:
                nc.sync.dma_start(
                    out=qT[:, (hg + 1) * 4 * TQ:(hg + 2) * 4 * TQ],
                    in_=d_qT[:, (hg + 1) * 4 * TQ:(hg + 2) * 4 * TQ])

        # V tiles: 4 direct + 2 gathered (PBC expansion), bf16
        v_sb = []
        for st in range(4):
            vt = const.tile([128, P * HID], BF16, tag=f"v{st}", name=f"v{st}")
            nc.sync.dma_start(out=vt[:], in_=d_vb[st * 128:(st + 1) * 128, :])
            v_sb.append(vt)
        idx_sb = const.tile([128, 2], I32, tag="idx")
        nc.gpsimd.dma_start(
            out=idx_sb[:].rearrange("p (two one) -> p two one", one=1),
            in_=d_vidx.rearrange("two p one -> p two one"))
        for gi in range(2):
            vt = const.tile([128, P * HID], BF16, tag=f"v{4 + gi}",
                            name=f"vg{gi}")
            nc.gpsimd.indirect_dma_start(
                out=vt[:], out_offset=None,
                in_=d_vb[:, :],
                in_offset=bass.IndirectOffsetOnAxis(
                    ap=idx_sb[:, gi:gi + 1], axis=0))
            v_sb.append(vt)

        # out_proj weights (needed only at the tail, but load mid-stream)
        nc.sync.dma_start(
            out=wT[:].rearrange("p (ci o) -> p ci o", ci=4),
            in_=d_wT.rearrange("(ci p) o -> p ci o", p=128))

        # SBUF workspace for the per-hg tail
        rec2 = const.tile([128, TQ], F32, tag="rec2")      # rows 0/32/64/96
        rec_sb = const.tile([128, TQ], F32, tag="rec_sb")  # bcast 32-blocks
        sq = const.tile([128, P * TQ], F32R, tag="sq")
        attn_ct = [attnp.tile([128, P * TQ], F16, tag=f"act{ci}",
                              name=f"act{ci}") for ci in range(4)]

        # PSUM layout (everything a whole number of banks, 8 total):
        #  scores pool (bufs=2): [128,1024] = 2 banks each      -> banks 0-3
        #  attn_ps [128,1024] (768 used): PV col-tiled accum    -> banks 4-5
        #  den [128,512]: rows{0,32,64,96} x [0:256]            -> bank 6
        #  ssq [128,512]: row0 x [0:256], exclusive bank so no
        #    foreign start=True clears its has_written bits     -> bank 7
        den_ps = psum1.tile([128, 512], F32, space="PSUM", tag="den")
        ssq_ps = psum1.tile([128, 512], F32, space="PSUM", tag="ssq")

        # ================= attention main loop =================
        for hg in range(4):
            attn_ps = psum1.tile([128, 1024], F32, space="PSUM", tag="attn",
                                 name=f"attn{hg}")
            # Zero the accumulation banks with full-array matmuls (sets
            # has_written everywhere; full-array MMs can't overlap col-tiled
            # ones, so no clear/write race). All den/PV matmuls then
            # accumulate with start=False.
            nc.tensor.matmul(out=attn_ps[:, 0:512], lhsT=ident[:],
                             rhs=zeros[:, 0:512], start=True, stop=True)
            nc.tensor.matmul(out=attn_ps[:, 512:1024], lhsT=ident[:],
                             rhs=zeros[:, 0:512], start=True, stop=True)
            nc.tensor.matmul(out=den_ps[:, 0:TQ], lhsT=ident[:],
                             rhs=zeros[:, 0:TQ], start=True, stop=True)
            for st in range(NST):
                scores = psum.tile([128, 4 * TQ], F32, space="PSUM",
                                   tag="scores")
                # bias preload into PSUM (identity matmul)
                bt = biasp.tile([128, 4 * TQ], F16, tag="bias")
                with (tc.high_priority() if (hg == 0 and st < 2)
                      else nullcontext()):
                    nc.sync.dma_start(
                        out=bt[:].rearrange("p (i t) -> p i t", i=4),
                        in_=d_bias[st * 128:(st + 1) * 128,
                                   hg * 4:hg * 4 + 4, :])
                for half in range(2):
                    nc.tensor.matmul(
                        out=scores[:, half * 512:(half + 1) * 512],
                        lhsT=ident[:],
                        rhs=bt[:, half * 512:(half + 1) * 512],
                        start=True, stop=False)
                # QK accumulate on top (i=1,3 close their banks)
                for i in range(4):
                    lo = st * (H * 128) + hg * 512 + i * 128
                    nc.tensor.matmul(
                        out=scores[:, i * TQ:(i + 1) * TQ],
                        lhsT=kst[:, lo:lo + 128],
                        rhs=qT[:, (hg * 4 + i) * TQ:(hg * 4 + i + 1) * TQ],
                        start=False, stop=(i % 2 == 1))
                # m = (scores + 20) * law    [one fat DVE pass]
                m = work.tile([128, 4 * TQ], F32, tag="m")
                law_st = law[:, st * TQ:(st + 1) * TQ]
                nc.vector.scalar_tensor_tensor(
                    out=m[:].rearrange("p (i t) -> p i t", i=4),
                    in0=scores[:].rearrange("p (i t) -> p i t", i=4),
                    scalar=SMOOTH,
                    in1=law_st.unsqueeze(1).to_broadcast([128, 4, TQ]),
                    op0=ALU.add, op1=ALU.mult)
                # e = exp(m - 20) -> bf16
                e = work.tile([128, 4 * TQ], BF16, tag="e")
                nc.scalar.activation(e[:], m[:], AF.Exp, bias=neg20[:],
                                     scale=1.0)
                # g = e * law (bf16); split DVE / gpsimd by load balance
                g = work.tile([128, 4 * TQ], BF16, tag="g")
                lawb_st = lawb[:, st * TQ:(st + 1) * TQ]
                geng = nc.vector if (hg * NST + st) % 3 == 0 else nc.gpsimd
                geng.tensor_tensor(
                    out=g[:].rearrange("p (i t) -> p i t", i=4),
                    in0=e[:].rearrange("p (i t) -> p i t", i=4),
                    in1=lawb_st.unsqueeze(1).to_broadcast([128, 4, TQ]),
                    op=ALU.mult)
                # denominators: 4 col-tiled ones-matmuls -> rows 32i
                for i in range(4):
                    nc.tensor.matmul(
                        out=den_ps[32 * i:32 * i + 1, 0:TQ],
                        lhsT=ones_b[:],
                        rhs=e[:, i * TQ:(i + 1) * TQ],
                        start=False, stop=(st == NST - 1),
                        tile_position=(0, 32 * i),
                        skip_group_check=True)
                # PV: col-tiled, writes channel-major attn directly
                for p in range(P):
                    for i in range(4):
                        h = hg * 4 + i
                        nc.tensor.matmul(
                            out=attn_ps[32 * i:32 * (i + 1),
                                        p * TQ:(p + 1) * TQ],
                            lhsT=v_sb[st][:, h * DH + p * 32:h * DH + p * 32 + 32],
                            rhs=g[:, i * TQ:(i + 1) * TQ],
                            start=False, stop=(st == NST - 1),
                            tile_position=(0, 32 * i),
                            skip_group_check=True)

            # ---- per-hg tail (overlaps next hg's attention) ----
            # recip over the whole den block (only rows 0/32/64/96 meaningful)
            nc.vector.reciprocal_approx_fast(
                out=rec2[:, :], in_=den_ps[:, 0:TQ])
            # broadcast each head's recip row over its 32 dd-partitions
            # (DRAM bounce: SBUF APs cannot have partition stride 0)
            nc.sync.dma_start(
                out=d_rec[hg],
                in_=rec2[0:97:32, :])
            for i in range(4):
                nc.scalar.dma_start(
                    out=rec_sb[32 * i:32 * (i + 1), :],
                    in_=d_rec[hg, i:i + 1, :].to_broadcast([32, TQ]))
            # eviction: attn_ct = attn_ps * rec  (divide fused, f16 out)
            nc.vector.tensor_tensor(
                out=attn_ct[hg][:].rearrange("c (p t) -> c p t", p=P),
                in0=attn_ps[:, 0:P * TQ].rearrange("c (p t) -> c p t", p=P),
                in1=rec_sb[:].unsqueeze(1).to_broadcast([128, P, TQ]),
                op=ALU.mult)
            # ssq accumulation for equivariant LN
            nc.scalar.activation(sq[:], attn_ct[hg][:], AF.Square,
                                 bias=0.0, scale=1.0)
            # ssq: fold p inside the accumulation group -> [1,256]
            for p in range(P):
                nc.tensor.matmul(out=ssq_ps[0:1, 0:TQ],
                                 lhsT=ones_f[:].bitcast(F32R),
                                 rhs=sq[:, p * TQ:(p + 1) * TQ],
                                 start=(hg == 0 and p == 0),
                                 stop=(hg == 3 and p == 2))

        # ================= equivariant LN scalars =================
        # inorm = rsqrt(ssq/512 + eps), one Newton step
        arow = const.tile([1, TQ], F32, tag="arow")
        nc.vector.tensor_scalar(
            out=arow[:], in0=ssq_ps[0:1, 0:TQ], scalar1=1.0 / HID,
            scalar2=EPS, op0=ALU.mult, op1=ALU.add)
        rcpa = const.tile([1, TQ], F32, tag="rcpa")
        nc.vector.reciprocal_approx_fast(out=rcpa[:], in_=arow[:])
        r0t = const.tile([1, TQ], F32, tag="r0t")
        nc.scalar.activation(r0t[:], rcpa[:], AF.Sqrt, bias=0.0, scale=1.0)
        tmp = const.tile([1, TQ], F32, tag="tmpn")
        nc.vector.tensor_tensor(out=tmp[:], in0=r0t[:], in1=r0t[:],
                                op=ALU.mult)
        nc.vector.tensor_tensor(out=tmp[:], in0=tmp[:], in1=arow[:],
                                op=ALU.mult)
        nc.vector.tensor_scalar(
            out=tmp[:], in0=tmp[:], scalar1=-0.5, scalar2=1.5,
            op0=ALU.mult, op1=ALU.add)
        inorm = const.tile([1, TQ], F32, tag="inorm")
        nc.vector.tensor_tensor(out=inorm[:], in0=r0t[:], in1=tmp[:],
                                op=ALU.mult)
        # inorm -> per-partition columns via K=1 PE broadcast (no DRAM bounce)
        icol_ps = psum1.tile([128, 2], F32, space="PSUM", tag="attn")
        for th in range(2):
            nc.tensor.matmul(
                out=icol_ps[:, th:th + 1],
                lhsT=inorm[0:1, th * 128:(th + 1) * 128],
                rhs=ones_f[0:1, 0:1],
                start=True, stop=True)
        icol = const.tile([128, 2], F32, tag="icol")
        nc.vector.tensor_copy(icol[:], icol_ps[:])

        # ================= out_proj =================
        for k in range(6):          # tp-tiles: p = k//2, t-half = k%2
            op = psum.tile([128, HID], F32, space="PSUM", tag="scores")
            for ci in range(4):
                nc.tensor.matmul(
                    out=op[:, :],
                    lhsT=attn_ct[ci][:, k * 128:(k + 1) * 128],
                    rhs=wT[:, ci * HID:(ci + 1) * HID],
                    start=(ci == 0), stop=(ci == 3))
            ot = work.tile([128, HID], F32, tag="osb")
            nc.scalar.activation(ot[:], op[:, :], AF.Copy,
                                 bias=0.0, scale=icol[:, k % 2:k % 2 + 1])
            nc.sync.dma_start(
                out=d_out[(k % 2) * 128:(k % 2) * 128 + 128, k // 2, :],
                in_=ot[:])


def _host_prep(q, k, v, attn_bias, local_attention_weight, out_proj_w,
               ln_weight, outcell_index):
    """Pure layout marshalling on host -> per-core input dicts."""
    q = np.asarray(q, np.float32)
    k = np.asarray(k, np.float32)
    v = np.asarray(v, np.float32)
    attn_bias = np.asarray(attn_bias, np.float32)
    law = np.asarray(local_attention_weight, np.float32)
    out_proj_w = np.asarray(out_proj_w, np.float32)
    ln_weight = np.asarray(ln_weight, np.float32)
    idx = np.asarray(outcell_index).astype(np.int64)

    # (B,T,P,HID) -> (B, 96, H, T) with row j = p*32+dd
    def to_dT(x):
        return np.ascontiguousarray(
            x.reshape(B, T, P, H, D).transpose(0, 2, 4, 3, 1)
        ).reshape(B, P * D, H, T)

    qT = to_dT(q) * np.float32(SCALING)
    kT = to_dT(k)
    # K PBC expansion along token axis (gather columns)
    kTe = np.concatenate(
        [kT, np.take_along_axis(
            kT, idx[:, None, None, :].astype(np.int64), axis=3)], axis=3)
    # reorder columns (h, s) -> (st, h, sd) for contiguous per-tile chunks
    kst = np.ascontiguousarray(
        kTe.reshape(B, DH, H, NST, 128).transpose(0, 1, 3, 2, 4)
    ).reshape(B, DH, NST * H * 128)
    biasT = np.ascontiguousarray(
        attn_bias.transpose(0, 3, 1, 2)).astype(np.float16)       # (B,S,H,T)
    lawT = np.ascontiguousarray(law.transpose(0, 2, 1))            # (B,S,T)
    lawTb = lawT.astype(ml_dtypes.bfloat16)
    # head-major V columns: (B, T, (h, p, dd)) so each head is contiguous
    vb = np.ascontiguousarray(
        v.reshape(B, T, P, H, D).transpose(0, 1, 3, 2, 4)
    ).reshape(B, T, P * HID).astype(ml_dtypes.bfloat16)
    wT = np.ascontiguousarray(out_proj_w.T) * ln_weight[:, None]   # (c,o)
    wT = np.ascontiguousarray(wT, np.float32).astype(np.float16)
    vidx = idx.astype(np.int32).reshape(B, 2, 128, 1)

    in_maps = []
    for c in range(NCORES):
        b, th = c // 2, c % 2
        t0 = th * TQ
        lawc = np.ascontiguousarray(lawT[b, :, t0:t0 + TQ])
        lawc = np.ascontiguousarray(
            lawc.reshape(NST, 128, TQ).transpose(1, 0, 2)).reshape(128, NST * TQ)
        lawcb = np.ascontiguousarray(lawTb[b, :, t0:t0 + TQ])
        lawcb = np.ascontiguousarray(
            lawcb.reshape(NST, 128, TQ).transpose(1, 0, 2)).reshape(128, NST * TQ)
        in_maps.append(dict(
            biasT=np.ascontiguousarray(biasT[b, :, :, t0:t0 + TQ]),
            lawT=lawc,
            lawTb=lawcb,
            qT=np.ascontiguousarray(
                qT[b, :, :, t0:t0 + TQ]).reshape(DH, H * TQ)
                .astype(ml_dtypes.bfloat16),
            kst=np.ascontiguousarray(kst[b]).astype(ml_dtypes.bfloat16),
            vb=np.ascontiguousarray(vb[b]),
            vidx=np.ascontiguousarray(vidx[b]),
            wT=wT,
            ident=np.eye(128, dtype=np.float16),
        ))
    return in_maps


def kernel(**inputs):
    global _CACHED_NC
    if _CACHED_NC is None:
        _CACHED_NC = build_nc()
    nc = _CACHED_NC
    in_maps = _host_prep(
        inputs["q"], inputs["k"], inputs["v"], inputs["attn_bias"],
        inputs["local_attention_weight"], inputs["out_proj_w"],
        inputs["ln_weight"], inputs["outcell_index"])
    res = run_bass_kernel_spmd(nc, in_maps, core_ids=list(range(NCORES)))
    out = np.empty((B, T, P, HID), np.float32)
    for c in range(NCORES):
        b, th = c // 2, c % 2
        out[b, th * TQ:(th + 1) * TQ] = res.results[c]["out"]
    return out


# revision 25
# speedup vs baseline: 1.1305x; 1.1305x over previous
"""Trainium2 Bass kernel for EquivariantAttention (sparse_attention).

Full (unsharded) inputs in, full output out. Internally shards over the 8
NeuronCores as (batch, T-half): core c handles batch b = c // 2, query rows
t0 = (c % 2) * 256 .. t0+256.  Every core runs the identical SPMD program on
its own input slices; there is no cross-core communication (LN and out_proj
are row-local in (b, t)).

Device-side per core:
  scores_T[s,(i,t)] = bias_T + (k_T.T @ q_T)      (bias preloaded into PSUM
                                                   via identity matmul, QK
                                                   accumulates on top)
  m = (scores_T + 20) * law_T                     (one DVE pass, fp32)
  e = exp(m - 20)  -> bf16                        (ACT, free affine bias)
  g = e * law_T                                   (DVE/gpsimd, bf16)
  den[i,t] = sum_s e                              (4 col-tiled ones-matmuls,
                                                   rows 32i of one PSUM bank)
  attn_ps[(i,dd),(p,t)] += v[s,(h,p,dd)].T @ g    (col-tiled matmuls write
                                                   channel-major attn directly
                                                   -> no head remap phase)
  attn_ct = attn_ps * recip(den)                  (divide fused into the PSUM
                                                   eviction, bcast via SBUF DMA)
  ssq accumulated via ones-matmuls on attn_ct^2   (ACT square)
  out = (attn_ct @ (out_proj_w * ln_w).T) * inorm (inorm cols via K=1 PE
                                                   broadcast, applied on ACT
                                                   PSUM->SBUF copy-out)
"""

import numpy as np
import ml_dtypes

import concourse.bass as bass
import concourse.bacc as bacc
import concourse.tile as tile
from concourse import mybir
from concourse.bass_utils import run_bass_kernel_spmd

# Problem constants (hardcoded per contract)
B, T, P, HID = 4, 512, 3, 512
H, D = 16, 32
EXP = 256
S = T + EXP            # 768
SCALING = (D / 3.0) ** 0.5 / D
SMOOTH = 20.0
EPS = 1e-3

NCORES = 8
TQ = T // 2            # 256 query rows per core
DH = P * D             # 96 head dim
NST = S // 128         # 6 s-tiles of 128
HG = 4                 # head groups of 4 heads

F32 = mybir.dt.float32
F16 = mybir.dt.float16
F32R = mybir.dt.float32r
BF16 = mybir.dt.bfloat16
I32 = mybir.dt.int32
AF = mybir.ActivationFunctionType
ALU = mybir.AluOpType

_CACHED_NC = None


def build_nc():
    nc = bacc.Bacc("TRN2", target_bir_lowering=False, debug=False)

    # ---- DRAM I/O (per-core shapes) ----
    d_bias = nc.dram_tensor("biasT", [S, H, TQ], F16, kind="ExternalInput").ap()
    d_law = nc.dram_tensor("lawT", [128, NST * TQ], F32, kind="ExternalInput").ap()
    d_lawb = nc.dram_tensor("lawTb", [128, NST * TQ], BF16, kind="ExternalInput").ap()
    d_qT = nc.dram_tensor("qT", [DH, H * TQ], BF16, kind="ExternalInput").ap()
    # K columns reordered (st, h, sd) so each (hg, st) chunk is contiguous
    d_kst = nc.dram_tensor("kst", [DH, NST * H * 128], BF16, kind="ExternalInput").ap()
    d_vb = nc.dram_tensor("vb", [T, P * HID], BF16, kind="ExternalInput").ap()
    d_vidx = nc.dram_tensor("vidx", [2, 128, 1], I32, kind="ExternalInput").ap()
    d_wT = nc.dram_tensor("wT", [HID, HID], F16, kind="ExternalInput").ap()
    d_id = nc.dram_tensor("ident", [128, 128], F16, kind="ExternalInput").ap()
    d_out = nc.dram_tensor("out", [TQ, P, HID], F32, kind="ExternalOutput").ap()
    d_rec = nc.dram_tensor("rec_scratch", [HG, 4, TQ], F32).ap()

    with tile.TileContext(nc) as tc:
        build_kernel(tc, d_bias, d_law, d_lawb, d_qT, d_kst, d_vb, d_vidx,
                     d_wT, d_id, d_out, d_rec)
    nc.compile()
    return nc


def build_kernel(tc, d_bias, d_law, d_lawb, d_qT, d_kst, d_vb, d_vidx,
                 d_wT, d_id, d_out, d_rec):
    nc = tc.nc
    from contextlib import ExitStack, nullcontext
    ctx = ExitStack()
    with ctx:
        const = ctx.enter_context(tc.tile_pool(name="const", bufs=1))
        big = ctx.enter_context(tc.tile_pool(name="big", bufs=1))
        biasp = ctx.enter_context(tc.tile_pool(name="biasp", bufs=2))
        work = ctx.enter_context(tc.tile_pool(name="work", bufs=3))
        attnp = ctx.enter_context(tc.tile_pool(name="attnp", bufs=1))
        psum = ctx.enter_context(tc.tile_pool(name="psum", bufs=2, space="PSUM"))
        psum1 = ctx.enter_context(tc.tile_pool(name="psum1", bufs=1, space="PSUM"))

        # ---- constants ----
        ident = const.tile([128, 128], F16, tag="ident")
        ones_b = const.tile([128, 1], BF16, tag="ones_b")
        ones_f = const.tile([128, 1], F32, tag="ones_f")
        neg20 = const.tile([128, 1], F32, tag="neg20")
        zeros = const.tile([128, 512], F16, tag="zeros")
        nc.vector.memset(ones_b[:], 1.0)
        nc.vector.memset(ones_f[:], 1.0)
        nc.vector.memset(neg20[:], -SMOOTH)
        nc.vector.memset(zeros[:], 0.0)

        # ---- resident tiles ----
        law = const.tile([128, NST * TQ], F32, tag="law")      # (s%128,(st,t))
        lawb = const.tile([128, NST * TQ], BF16, tag="lawb")
        qT = const.tile([DH, H * TQ], BF16, tag="qT")
        kst = big.tile([DH, NST * H * 128], BF16, tag="kst")   # (d,(st,h,sd))
        wT = const.tile([128, 4 * HID], F16, tag="wT")         # (c%128,(ci,o))

        def kst_hg_dma(hg):
            # one 3D-AP DMA per head-group: 6 strided [96, 512] chunks
            ov = kst[:].rearrange("d (st h sd) -> d st h sd", st=NST, h=H)
            iv = d_kst.rearrange("d (st h sd) -> d st h sd", st=NST, h=H)
            nc.sync.dma_start(out=ov[:, :, hg * 4:(hg + 1) * 4, :],
                              in_=iv[:, :, hg * 4:(hg + 1) * 4, :])

        # need-ordered startup: first-tile inputs at high priority,
        # few fat DMAs (SP descriptor-gen is ~600ns/instruction, serial)
        with tc.high_priority():
            nc.sync.dma_start(out=ident[:], in_=d_id)
            nc.sync.dma_start(out=law[:], in_=d_law)
            kst_hg_dma(0)
            nc.sync.dma_start(out=qT[:], in_=d_qT)
        # V tiles: 4 direct + 2 gathered (PBC expansion), bf16
        v_sb = []
        for st in range(4):
            vt = const.tile([128, P * HID], BF16, tag=f"v{st}", name=f"v{st}")
            nc.sync.dma_start(out=vt[:], in_=d_vb[st * 128:(st + 1) * 128, :])
            v_sb.append(vt)
        idx_sb = const.tile([128, 2], I32, tag="idx")
        nc.gpsimd.dma_start(
            out=idx_sb[:].rearrange("p (two one) -> p two one", one=1),
            in_=d_vidx.rearrange("two p one -> p two one"))
        for gi in range(2):
            vt = const.tile([128, P * HID], BF16, tag=f"v{4 + gi}",
                            name=f"vg{gi}")
            nc.gpsimd.indirect_dma_start(
                out=vt[:], out_offset=None,
                in_=d_vb[:, :],
                in_offset=bass.IndirectOffsetOnAxis(
                    ap=idx_sb[:, gi:gi + 1], axis=0))
            v_sb.append(vt)
        nc.scalar.dma_start(out=lawb[:], in_=d_lawb)
        for hg in range(1, HG):
            kst_hg_dma(hg)

        # out_proj weights (needed only at the tail, but load mid-stream)
        nc.sync.dma_start(
            out=wT[:].rearrange("p (ci o) -> p ci o", ci=4),
            in_=d_wT.rearrange("(ci p) o -> p ci o", p=128))

        # SBUF workspace for the per-hg tail
        rec2 = const.tile([128, TQ], F32, tag="rec2")      # rows 0/32/64/96
        rec_sb = const.tile([128, TQ], F32, tag="rec_sb")  # bcast 32-blocks
        sq = const.tile([128, P * TQ], F32R, tag="sq")
        attn_ct = [attnp.tile([128, P * TQ], F16, tag=f"act{ci}",
                              name=f"act{ci}") for ci in range(4)]

        # PSUM layout (everything a whole number of banks, 8 total):
        #  scores pool (bufs=2): [128,1024] = 2 banks each      -> banks 0-3
        #  attn_ps [128,1024] (768 used): PV col-tiled accum    -> banks 4-5
        #  den [128,512]: rows{0,32,64,96} x [0:256]            -> bank 6
        #  ssq [128,512]: row0 x [0:256], exclusive bank so no
        #    foreign start=True clears its has_written bits     -> bank 7
        den_ps = psum1.tile([128, 512], F32, space="PSUM", tag="den")
        ssq_ps = psum1.tile([128, 512], F32, space="PSUM", tag="ssq")

        # ================= attention main loop =================
        for hg in range(4):
            attn_ps = psum1.tile([128, 1024], F32, space="PSUM", tag="attn",
                                 name=f"attn{hg}")
            # Zero the accumulation banks with full-array matmuls (sets
            # has_written everywhere; full-array MMs can't overlap col-tiled
            # ones, so no clear/write race). All den/PV matmuls then
            # accumulate with start=False.
            nc.tensor.matmul(out=attn_ps[:, 0:512], lhsT=ident[:],
                             rhs=zeros[:, 0:512], start=True, stop=True)
            nc.tensor.matmul(out=attn_ps[:, 512:1024], lhsT=ident[:],
                             rhs=zeros[:, 0:512], start=True, stop=True)
            nc.tensor.matmul(out=den_ps[:, 0:TQ], lhsT=ident[:],
                             rhs=zeros[:, 0:TQ], start=True, stop=True)
            # bias for the whole head-group in 1-2 DMAs (scalar queue)
            bth = biasp.tile([128, NST * 1024], F16, tag="bias")

            def bias_dma(st0, st1, pri):
                with (tc.high_priority() if pri else nullcontext()):
                    nc.scalar.dma_start(
                        out=bth[:, st0 * 1024:st1 * 1024].rearrange(
                            "p (st i t) -> p st i t", i=4, t=TQ),
                        in_=d_bias[st0 * 128:st1 * 128, hg * 4:hg * 4 + 4, :]
                            .rearrange("(st p) i t -> p st i t", p=128))
            if hg == 0:
                bias_dma(0, 1, True)
                bias_dma(1, NST, False)
            else:
                bias_dma(0, NST, False)
            for st in range(NST):
                scores = psum.tile([128, 4 * TQ], F32, space="PSUM",
                                   tag="scores")
                # bias preload into PSUM (identity matmul)
                bt = bth[:, st * 1024:(st + 1) * 1024]
                for half in range(2):
                    nc.tensor.matmul(
                        out=scores[:, half * 512:(half + 1) * 512],
                        lhsT=ident[:],
                        rhs=bt[:, half * 512:(half + 1) * 512],
                        start=True, stop=False)
                # QK accumulate on top (i=1,3 close their banks)
                for i in range(4):
                    lo = st * (H * 128) + hg * 512 + i * 128
                    nc.tensor.matmul(
                        out=scores[:, i * TQ:(i + 1) * TQ],
                        lhsT=kst[:, lo:lo + 128],
                        rhs=qT[:, (hg * 4 + i) * TQ:(hg * 4 + i + 1) * TQ],
                        start=False, stop=(i % 2 == 1))
                # m = (scores + 20) * law    [one fat DVE pass]
                m = work.tile([128, 4 * TQ], F32, tag="m")
                law_st = law[:, st * TQ:(st + 1) * TQ]
                nc.vector.scalar_tensor_tensor(
                    out=m[:].rearrange("p (i t) -> p i t", i=4),
                    in0=scores[:].rearrange("p (i t) -> p i t", i=4),
                    scalar=SMOOTH,
                    in1=law_st.unsqueeze(1).to_broadcast([128, 4, TQ]),
                    op0=ALU.add, op1=ALU.mult)
                # e = exp(m - 20) -> bf16
                e = work.tile([128, 4 * TQ], BF16, tag="e")
                nc.scalar.activation(e[:], m[:], AF.Exp, bias=neg20[:],
                                     scale=1.0)
                # g = e * law (bf16); split DVE / gpsimd by load balance
                g = work.tile([128, 4 * TQ], BF16, tag="g")
                lawb_st = lawb[:, st * TQ:(st + 1) * TQ]
                geng = nc.vector if (hg * NST + st) % 3 == 0 else nc.gpsimd
                geng.tensor_tensor(
                    out=g[:].rearrange("p (i t) -> p i t", i=4),
                    in0=e[:].rearrange("p (i t) -> p i t", i=4),
                    in1=lawb_st.unsqueeze(1).to_broadcast([128, 4, TQ]),
                    op=ALU.mult)
                # denominators: 4 col-tiled ones-matmuls -> rows 32i
                for i in range(4):
                    nc.tensor.matmul(
                        out=den_ps[32 * i:32 * i + 1, 0:TQ],
                        lhsT=ones_b[:],
                        rhs=e[:, i * TQ:(i + 1) * TQ],
                        start=False, stop=(st == NST - 1),
                        tile_position=(0, 32 * i),
                        skip_group_check=True)
                # PV: col-tiled, writes channel-major attn directly
                for p in range(P):
                    for i in range(4):
                        h = hg * 4 + i
                        nc.tensor.matmul(
                            out=attn_ps[32 * i:32 * (i + 1),
                                        p * TQ:(p + 1) * TQ],
                            lhsT=v_sb[st][:, h * DH + p * 32:h * DH + p * 32 + 32],
                            rhs=g[:, i * TQ:(i + 1) * TQ],
                            start=False, stop=(st == NST - 1),
                            tile_position=(0, 32 * i),
                            skip_group_check=True)

            # ---- per-hg tail (overlaps next hg's attention) ----
            # recip over the whole den block (only rows 0/32/64/96 meaningful)
            nc.vector.reciprocal_approx_fast(
                out=rec2[:, :], in_=den_ps[:, 0:TQ])
            # broadcast each head's recip row over its 32 dd-partitions
            # (DRAM bounce: SBUF APs cannot have partition stride 0)
            nc.sync.dma_start(
                out=d_rec[hg],
                in_=rec2[0:97:32, :])
            for i in range(4):
                nc.scalar.dma_start(
                    out=rec_sb[32 * i:32 * (i + 1), :],
                    in_=d_rec[hg, i:i + 1, :].to_broadcast([32, TQ]))
            # eviction: attn_ct = attn_ps * rec  (divide fused, f16 out)
            nc.vector.tensor_tensor(
                out=attn_ct[hg][:].rearrange("c (p t) -> c p t", p=P),
                in0=attn_ps[:, 0:P * TQ].rearrange("c (p t) -> c p t", p=P),
                in1=rec_sb[:].unsqueeze(1).to_broadcast([128, P, TQ]),
                op=ALU.mult)
            # ssq accumulation for equivariant LN
            nc.scalar.activation(sq[:], attn_ct[hg][:], AF.Square,
                                 bias=0.0, scale=1.0)
            # ssq: fold p inside the accumulation group -> [1,256]
            for p in range(P):
                nc.tensor.matmul(out=ssq_ps[0:1, 0:TQ],
                                 lhsT=ones_f[:].bitcast(F32R),
                                 rhs=sq[:, p * TQ:(p + 1) * TQ],
                                 start=(hg == 0 and p == 0),
                                 stop=(hg == 3 and p == 2))

        # ================= equivariant LN scalars =================
        # inorm = rsqrt(ssq/512 + eps), one Newton step
        arow = const.tile([1, TQ], F32, tag="arow")
        nc.vector.tensor_scalar(
            out=arow[:], in0=ssq_ps[0:1, 0:TQ], scalar1=1.0 / HID,
            scalar2=EPS, op0=ALU.mult, op1=ALU.add)
        rcpa = const.tile([1, TQ], F32, tag="rcpa")
        nc.vector.reciprocal_approx_fast(out=rcpa[:], in_=arow[:])
        r0t = const.tile([1, TQ], F32, tag="r0t")
        nc.scalar.activation(r0t[:], rcpa[:], AF.Sqrt, bias=0.0, scale=1.0)
        tmp = const.tile([1, TQ], F32, tag="tmpn")
        nc.vector.tensor_tensor(out=tmp[:], in0=r0t[:], in1=r0t[:],
                                op=ALU.mult)
        nc.vector.tensor_tensor(out=tmp[:], in0=tmp[:], in1=arow[:],
                                op=ALU.mult)
        nc.vector.tensor_scalar(
            out=tmp[:], in0=tmp[:], scalar1=-0.5, scalar2=1.5,
            op0=ALU.mult, op1=ALU.add)
        inorm = const.tile([1, TQ], F32, tag="inorm")
        nc.vector.tensor_tensor(out=inorm[:], in0=r0t[:], in1=tmp[:],
                                op=ALU.mult)
        # inorm -> per-partition columns via K=1 PE broadcast (no DRAM bounce)
        icol_ps = psum1.tile([128, 2], F32, space="PSUM", tag="attn")
        for th in range(2):
            nc.tensor.matmul(
                out=icol_ps[:, th:th + 1],
                lhsT=inorm[0:1, th * 128:(th + 1) * 128],
                rhs=ones_f[0:1, 0:1],
                start=True, stop=True)
        icol = const.tile([128, 2], F32, tag="icol")
        nc.vector.tensor_copy(icol[:], icol_ps[:])

        # ================= out_proj =================
        for k in range(6):          # tp-tiles: p = k//2, t-half = k%2
            op = psum.tile([128, HID], F32, space="PSUM", tag="scores")
            for ci in range(4):
                nc.tensor.matmul(
                    out=op[:, :],
                    lhsT=attn_ct[ci][:, k * 128:(k + 1) * 128],
                    rhs=wT[:, ci * HID:(ci + 1) * HID],
                    start=(ci == 0), stop=(ci == 3))
            ot = work.tile([128, HID], F32, tag="osb")
            nc.scalar.activation(ot[:], op[:, :], AF.Copy,
                                 bias=0.0, scale=icol[:, k % 2:k % 2 + 1])
            nc.sync.dma_start(
                out=d_out[(k % 2) * 128:(k % 2) * 128 + 128, k // 2, :],
                in_=ot[:])


def _host_prep(q, k, v, attn_bias, local_attention_weight, out_proj_w,
               ln_weight, outcell_index):
    """Pure layout marshalling on host -> per-core input dicts."""
    q = np.asarray(q, np.float32)
    k = np.asarray(k, np.float32)
    v = np.asarray(v, np.float32)
    attn_bias = np.asarray(attn_bias, np.float32)
    law = np.asarray(local_attention_weight, np.float32)
    out_proj_w = np.asarray(out_proj_w, np.float32)
    ln_weight = np.asarray(ln_weight, np.float32)
    idx = np.asarray(outcell_index).astype(np.int64)

    # (B,T,P,HID) -> (B, 96, H, T) with row j = p*32+dd
    def to_dT(x):
        return np.ascontiguousarray(
            x.reshape(B, T, P, H, D).transpose(0, 2, 4, 3, 1)
        ).reshape(B, P * D, H, T)

    qT = to_dT(q) * np.float32(SCALING)
    kT = to_dT(k)
    # K PBC expansion along token axis (gather columns)
    kTe = np.concatenate(
        [kT, np.take_along_axis(
            kT, idx[:, None, None, :].astype(np.int64), axis=3)], axis=3)
    # reorder columns (h, s) -> (st, h, sd) for contiguous per-tile chunks
    kst = np.ascontiguousarray(
        kTe.reshape(B, DH, H, NST, 128).transpose(0, 1, 3, 2, 4)
    ).reshape(B, DH, NST * H * 128)
    biasT = np.ascontiguousarray(
        attn_bias.transpose(0, 3, 1, 2)).astype(np.float16)       # (B,S,H,T)
    lawT = np.ascontiguousarray(law.transpose(0, 2, 1))            # (B,S,T)
    lawTb = lawT.astype(ml_dtypes.bfloat16)
    # head-major V columns: (B, T, (h, p, dd)) so each head is contiguous
    vb = np.ascontiguousarray(
        v.reshape(B, T, P, H, D).transpose(0, 1, 3, 2, 4)
    ).reshape(B, T, P * HID).astype(ml_dtypes.bfloat16)
    wT = np.ascontiguousarray(out_proj_w.T) * ln_weight[:, None]   # (c,o)
    wT = np.ascontiguousarray(wT, np.float32).astype(np.float16)
    vidx = idx.astype(np.int32).reshape(B, 2, 128, 1)

    in_maps = []
    for c in range(NCORES):
        b, th = c // 2, c % 2
        t0 = th * TQ
        lawc = np.ascontiguousarray(lawT[b, :, t0:t0 + TQ])
        lawc = np.ascontiguousarray(
            lawc.reshape(NST, 128, TQ).transpose(1, 0, 2)).reshape(128, NST * TQ)
        lawcb = np.ascontiguousarray(lawTb[b, :, t0:t0 + TQ])
        lawcb = np.ascontiguousarray(
            lawcb.reshape(NST, 128, TQ).transpose(1, 0, 2)).reshape(128, NST * TQ)
        in_maps.append(dict(
            biasT=np.ascontiguousarray(biasT[b, :, :, t0:t0 + TQ]),
            lawT=lawc,
            lawTb=lawcb,
            qT=np.ascontiguousarray(
                qT[b, :, :, t0:t0 + TQ]).reshape(DH, H * TQ)
                .astype(ml_dtypes.bfloat16),
            kst=np.ascontiguousarray(kst[b]).astype(ml_dtypes.bfloat16),
            vb=np.ascontiguousarray(vb[b]),
            vidx=np.ascontiguousarray(vidx[b]),
            wT=wT,
            ident=np.eye(128, dtype=np.float16),
        ))
    return in_maps


def kernel(**inputs):
    global _CACHED_NC
    if _CACHED_NC is None:
        _CACHED_NC = build_nc()
    nc = _CACHED_NC
    in_maps = _host_prep(
        inputs["q"], inputs["k"], inputs["v"], inputs["attn_bias"],
        inputs["local_attention_weight"], inputs["out_proj_w"],
        inputs["ln_weight"], inputs["outcell_index"])
    res = run_bass_kernel_spmd(nc, in_maps, core_ids=list(range(NCORES)))
    out = np.empty((B, T, P, HID), np.float32)
    for c in range(NCORES):
        b, th = c // 2, c % 2
        out[b, th * TQ:(th + 1) * TQ] = res.results[c]["out"]
    return out
